# revision 16
# baseline (speedup 1.0000x reference)
"""Trainium2 Bass kernel for nn_Critic (branch MLPs -> 255-step LSTM -> head).

Strategy (hardcoded, 8 cores, data-parallel over batch B=512 -> 64/core):
  - Feature-major on chip: vectors are [feature_chunk(128), batch(64)].
  - bf16 matmul inputs, fp32 PSUM/gates/cell state.
  - PSUM z_all[p, gate, slot, mmcol]: gate-major (weights permuted to
    f,i,g,o order; g rows pre-scaled by 2 so tanh(zg) = 2*sigmoid(2zg)-1).
    Each gate owns 2 banks (8 slots of 128 cols); slot = t mod 8. sigmoid(f)
    for step t can run while the PE still writes i/g/o of the same step
    (different banks), shortening the serial gate chain.
  - Gate chain per step: sig_f -> sig_ig -> sig_o on ACT; on DVE
    tm2 = sf*c, tm1 = (sg-0.5)*si (scalar_tensor_tensor), c' = 2*tm1+tm2
    (scalar_tensor_tensor), then tanh(c') on ACT, h = so*tc on DVE.
  - zx (Wk^T x_t) for step t+4 is emitted right after step t's recurrent
    matmuls: always lands in the opposite bank-half from the sigmoids of
    steps t..t+3, so no PSUM bank serialization, and the PE stays warm
    (no HAM re-throttle). start=True (whole-bank clear) only on slot 0/4
    of each bank, whose other slots hold only dead data at that point.
"""

import os
os.environ.setdefault("TILE_EXHAUSTIVE_MEMORY_SHARE_CHECK", "1")

import numpy as np
import ml_dtypes

import concourse.bass as bass
import concourse.mybir as mybir
import concourse.tile as tile
from concourse import bacc
from concourse.bass_utils import run_bass_kernel_spmd

BF16 = mybir.dt.bfloat16
F32 = mybir.dt.float32
AF = mybir.ActivationFunctionType
ALU = mybir.AluOpType

NC = 8          # cores
B = 512
BC = B // NC    # 64 batch per core
T = 255         # real steps
TP = 256        # padded steps
U = 256
DIN = 256
ND = 5          # PE-warming dummy matmuls (N=512) per step


def build_nc(use_bias=False):
    nc = bacc.Bacc(None, target_bir_lowering=False)

    d_mot = nc.dram_tensor("mot", [64, BC], BF16, kind="ExternalInput")
    d_rob = nc.dram_tensor("rob", [128, BC], BF16, kind="ExternalInput")
    d_re = nc.dram_tensor("re_", [128, BC], BF16, kind="ExternalInput")
    d_im = nc.dram_tensor("im_", [128, BC], BF16, kind="ExternalInput")
    d_seq = nc.dram_tensor("seq", [2, 128, TP * BC], BF16, kind="ExternalInput")
    d_wm = nc.dram_tensor("wm", [64, 256], BF16, kind="ExternalInput")
    d_wr = nc.dram_tensor("wr", [128, 256], BF16, kind="ExternalInput")
    d_wre = nc.dram_tensor("wre", [128, 128], BF16, kind="ExternalInput")
    d_wim = nc.dram_tensor("wim", [128, 128], BF16, kind="ExternalInput")
    d_wc = nc.dram_tensor("wc", [128, 6, 256], BF16, kind="ExternalInput")
    d_wk = nc.dram_tensor("wk", [128, 2, 1024], BF16, kind="ExternalInput")
    d_wrk = nc.dram_tensor("wrk", [128, 2, 1024], BF16, kind="ExternalInput")
    d_wo = nc.dram_tensor("wo", [128, 2, 1], BF16, kind="ExternalInput")
    d_bm = nc.dram_tensor("bm2", [128, 2], F32, kind="ExternalInput")
    d_br = nc.dram_tensor("br2", [128, 2], F32, kind="ExternalInput")
    d_bre = nc.dram_tensor("bre1", [128, 1], F32, kind="ExternalInput")
    d_bim = nc.dram_tensor("bim1", [128, 1], F32, kind="ExternalInput")
    d_bc = nc.dram_tensor("bc2", [128, 2], F32, kind="ExternalInput")
    d_bo = nc.dram_tensor("bo1", [1, 1], F32, kind="ExternalInput")
    d_blt = nc.dram_tensor("blt", [128, 8], F32, kind="ExternalInput")
    d_y = nc.dram_tensor("y", [1, BC], F32, kind="ExternalOutput")

    with tile.TileContext(nc) as tc:
        with (
            tc.tile_pool(name="sb", bufs=1) as sb,
            tc.tile_pool(name="rot", bufs=3) as rot,
        ):
            t_wk = sb.tile([128, 2, 1024], BF16, tag="wk")
            t_wrk = sb.tile([128, 2, 1024], BF16, tag="wrk")
            t_blt = sb.tile([128, 8], F32, tag="blt")
            t_seq0 = sb.tile([128, TP * BC], BF16, tag="seq0")
            t_seq1 = sb.tile([128, TP * BC], BF16, tag="seq1")
            t_wm = sb.tile([64, 256], BF16, tag="wm")
            t_wr = sb.tile([128, 256], BF16, tag="wr")
            t_wre = sb.tile([128, 128], BF16, tag="wre")
            t_wim = sb.tile([128, 128], BF16, tag="wim")
            t_wc = sb.tile([128, 6, 256], BF16, tag="wc")
            t_wo = sb.tile([128, 2, 1], BF16, tag="wo")
            t_mot = sb.tile([64, BC], BF16, tag="mot")
            t_rob = sb.tile([128, BC], BF16, tag="rob")
            t_re = sb.tile([128, BC], BF16, tag="re")
            t_im = sb.tile([128, BC], BF16, tag="im")
            t_bm = sb.tile([128, 2], F32, tag="bm")
            t_br = sb.tile([128, 2], F32, tag="br")
            t_bre = sb.tile([128, 1], F32, tag="bre")
            t_bim = sb.tile([128, 1], F32, tag="bim")
            t_bc = sb.tile([128, 2], F32, tag="bc")
            t_bo = sb.tile([1, 1], F32, tag="bo")
            t_h = sb.tile([128, 2 * BC], BF16, tag="h")   # h^T (chunk k at cols k*64)
            t_c = sb.tile([128, 2 * BC], F32, tag="c")    # c^T
            t_z512 = sb.tile([128, 512], BF16, tag="z512")  # zero rhs for PE-warming
            t_cat = sb.tile([128, 6, BC], BF16, tag="cat")
            t_y = sb.tile([1, BC], F32, tag="y")

            # Input DMAs spread across engine queues so the ~0.7us/descriptor
            # issue cost doesn't serialize on one engine. The first seq chunk
            # (steps 0..63) goes first -- it gates zx(0..3).
            CH = 64 * BC
            nc.sync.dma_start(t_seq0[:, 0:CH], d_seq[0, :, 0:CH])
            nc.gpsimd.dma_start(t_seq1[:, 0:CH], d_seq[1, :, 0:CH])
            # front-end + LSTM weights on the scalar queue (+ sync/gpsimd
            # after their first seq chunk)
            nc.scalar.dma_start(t_wm[:], d_wm[:])
            nc.scalar.dma_start(t_wr[:], d_wr[:])
            nc.scalar.dma_start(t_wre[:], d_wre[:])
            nc.scalar.dma_start(t_wim[:], d_wim[:])
            nc.scalar.dma_start(t_mot[:], d_mot[:])
            nc.scalar.dma_start(t_rob[:], d_rob[:])
            nc.scalar.dma_start(t_re[:], d_re[:])
            nc.scalar.dma_start(t_im[:], d_im[:])
            nc.sync.dma_start(t_wc[:], d_wc[:])
            nc.gpsimd.dma_start(t_wk[:], d_wk[:])
            nc.sync.dma_start(t_wrk[:], d_wrk[:])
            nc.scalar.dma_start(t_bm[:], d_bm[:])
            nc.scalar.dma_start(t_br[:], d_br[:])
            nc.scalar.dma_start(t_bre[:], d_bre[:])
            nc.scalar.dma_start(t_bim[:], d_bim[:])
            nc.scalar.dma_start(t_bc[:], d_bc[:])
            nc.scalar.dma_start(t_bo[:], d_bo[:])
            nc.scalar.dma_start(t_blt[:], d_blt[:])
            nc.scalar.dma_start(t_wo[:], d_wo[:])
            for ch in range(1, TP * BC // CH):
                nc.sync.dma_start(
                    t_seq0[:, ch * CH:(ch + 1) * CH], d_seq[0, :, ch * CH:(ch + 1) * CH])
                nc.gpsimd.dma_start(
                    t_seq1[:, ch * CH:(ch + 1) * CH], d_seq[1, :, ch * CH:(ch + 1) * CH])
            t_seq = [t_seq0, t_seq1]
            nc.vector.memset(t_z512[:], 0.0)

            # ---- front-end branch MLPs -> state -> h0, c0 ----
            with tc.tile_pool(name="fp", bufs=1, space="PSUM") as fp:
                p6 = fp.tile([128, 6, BC], F32, tag="p6")
                for m in range(2):
                    nc.tensor.matmul(p6[:, m, :], t_wm[:, m * 128:(m + 1) * 128],
                                     t_mot[:], start=True, stop=True)
                for m in range(2):
                    nc.tensor.matmul(p6[:, 2 + m, :], t_wr[:, m * 128:(m + 1) * 128],
                                     t_rob[:], start=True, stop=True)
                nc.tensor.matmul(p6[:, 4, :], t_wre[:], t_re[:], start=True, stop=True)
                nc.tensor.matmul(p6[:, 5, :], t_wim[:], t_im[:], start=True, stop=True)
                for m in range(2):
                    nc.scalar.activation(t_cat[:, m, :], p6[:, m, :], AF.Relu,
                                         bias=t_bm[:, m:m + 1])
                for m in range(2):
                    nc.scalar.activation(t_cat[:, 2 + m, :], p6[:, 2 + m, :], AF.Relu,
                                         bias=t_br[:, m:m + 1])
                nc.scalar.activation(t_cat[:, 4, :], p6[:, 4, :], AF.Relu,
                                     bias=t_bre[:, 0:1])
                nc.scalar.activation(t_cat[:, 5, :], p6[:, 5, :], AF.Relu,
                                     bias=t_bim[:, 0:1])
                pst = fp.tile([128, 2, BC], F32, tag="pst")
                for mo in range(2):
                    for kc in range(6):
                        nc.tensor.matmul(
                            pst[:, mo, :],
                            t_wc[:, kc, mo * 128:(mo + 1) * 128],
                            t_cat[:, kc, :],
                            start=(kc == 0), stop=(kc == 5))
                for mo in range(2):
                    nc.scalar.activation(t_h[:, mo * BC:(mo + 1) * BC], pst[:, mo, :],
                                         AF.Relu, bias=t_bc[:, mo:mo + 1])
                    nc.scalar.activation(t_c[:, mo * BC:(mo + 1) * BC], pst[:, mo, :],
                                         AF.Relu, bias=t_bc[:, mo:mo + 1])

            # ---- LSTM recurrence ----
            with tc.tile_pool(name="zp", bufs=1, space="PSUM") as zp:
                # z_all[p, gate, slot, mm*64+b]: gate order f,i,g,o (weights
                # permuted; g pre-scaled x2). Each gate = 2 banks; slot = t%8.
                z_all = zp.tile([128, 4, 8, 128], F32, tag="zall")

                def emit_zx(tp):
                    # zx for step tp: 16 MMs; start=True (whole-bank clear)
                    # on the first MM into EACH gate's bank at the quad
                    # boundary (tp%4==0) -- that bank's other slots hold only
                    # already-consumed steps then.
                    for g in range(4):
                        for mm in range(2):
                            for k in range(2):
                                st = (tp % 4 == 0 and mm == 0 and k == 0)
                                nc.tensor.matmul(
                                    z_all[:, g, tp % 8, mm * BC:(mm + 1) * BC],
                                    t_wk[:, k, (g * 2 + mm) * 128:(g * 2 + mm + 1) * 128],
                                    t_seq[k][:, tp * BC:(tp + 1) * BC],
                                    start=st, stop=False,
                                    skip_group_check=True)

                def emit_step(t):
                    s = t % 8
                    # recurrent matmuls, gate-major so sigmoid(f) can start
                    # after the first 4 MMs (f banks are done being written)
                    for g in range(4):
                        for mm in range(2):
                            for k in range(2):
                                nc.tensor.matmul(
                                    z_all[:, g, s, mm * BC:(mm + 1) * BC],
                                    t_wrk[:, k, (g * 2 + mm) * 128:(g * 2 + mm + 1) * 128],
                                    t_h[:, k * BC:(k + 1) * BC],
                                    start=False,
                                    stop=(mm == 1 and k == 1),
                                    skip_group_check=True)
                    gbf = rot.tile([128, 128], F32, tag="gbf")    # sigma_f
                    gbig = rot.tile([128, 2, 128], BF16, tag="gbig")  # si, sg
                    gbo = rot.tile([128, 128], BF16, tag="gbo")   # sigma_o
                    tm1 = rot.tile([128, 128], BF16, tag="tm1")
                    tm2 = rot.tile([128, 128], F32, tag="tm2")
                    tmc = rot.tile([128, 128], BF16, tag="tmc")
                    if not use_bias:
                        nc.scalar.activation(gbf[:], z_all[:, 0, s, :],
                                             AF.Sigmoid)
                        nc.scalar.activation(gbig[:], z_all[:, 1:3, s, :],
                                             AF.Sigmoid)
                        nc.scalar.activation(gbo[:], z_all[:, 3, s, :],
                                             AF.Sigmoid)
                    else:
                        # general-bias fallback: per-chunk sigmoids with the
                        # per-partition bias column (g chunks carry 2*bl).
                        for mm in range(2):
                            nc.scalar.activation(
                                gbf[:, mm * BC:(mm + 1) * BC],
                                z_all[:, 0, s, mm * BC:(mm + 1) * BC],
                                AF.Sigmoid, bias=t_blt[:, mm:mm + 1])
                        for gi in range(2):
                            for mm in range(2):
                                nc.scalar.activation(
                                    gbig[:, gi, mm * BC:(mm + 1) * BC],
                                    z_all[:, 1 + gi, s, mm * BC:(mm + 1) * BC],
                                    AF.Sigmoid,
                                    bias=t_blt[:, 2 + gi * 2 + mm:3 + gi * 2 + mm])
                        for mm in range(2):
                            nc.scalar.activation(
                                gbo[:, mm * BC:(mm + 1) * BC],
                                z_all[:, 3, s, mm * BC:(mm + 1) * BC],
                                AF.Sigmoid, bias=t_blt[:, 6 + mm:7 + mm])
                    # tm2 = sf*c ; tm1 = (sg-0.5)*si ; c' = 2*tm1 + tm2
                    nc.vector.tensor_mul(tm2[:], gbf[:], t_c[:])
                    nc.vector.scalar_tensor_tensor(
                        tm1[:], gbig[:, 1, :], -0.5, gbig[:, 0, :],
                        op0=ALU.add, op1=ALU.mult)
                    nc.vector.scalar_tensor_tensor(
                        t_c[:], tm1[:], 2.0, tm2[:],
                        op0=ALU.mult, op1=ALU.add)
                    nc.scalar.activation(tmc[:], t_c[:], AF.Tanh)
                    nc.vector.tensor_mul(t_h[:], gbo[:], tmc[:])

                    # zx for step t+4 + PE-warming matmuls, emitted AFTER the
                    # gate ops so the scheduler keeps the recurrent MMs and
                    # gate chain tight; these fill the PE during the next
                    # step's gate chain. zx banks are always in the opposite
                    # slot-half from the sigmoid reads of steps t..t+3.
                    # Warming MMs stream a zero rhs with start=False -- adds 0
                    # everywhere (or writes 0 that real zx then accumulates
                    # onto), keeping PE duty high so HAM stays at 8/8.
                    if t + 4 < T:
                        emit_zx(t + 4)
                    H = 4 * (((t + 4) % 8) // 4)
                    for dj in range(ND):
                        nc.tensor.matmul(
                            z_all[:, dj % 4, H:H + 4, :],
                            t_wk[:, 0, (dj % 8) * 128:(dj % 8 + 1) * 128],
                            t_z512[:],
                            start=False, stop=False, skip_group_check=True)

                for tp in range(4):
                    emit_zx(tp)
                for t in range(T):
                    emit_step(t)

            # ---- output head ----
            with tc.tile_pool(name="hp", bufs=1, space="PSUM") as hp:
                py = hp.tile([1, BC], F32, tag="py")
                for k in range(2):
                    nc.tensor.matmul(py[:], t_wo[:, k, :], t_h[:, k * BC:(k + 1) * BC],
                                     start=(k == 0), stop=(k == 1))
                nc.scalar.activation(t_y[:], py[:], AF.Relu, bias=t_bo[:, 0:1])
            nc.sync.dma_start(d_y[:], t_y[:])

    nc.compile()
    return nc


_NC_CACHE = None


def _prep_inputs(inputs):
    """Shard + lay out the full-problem inputs into 8 per-core in_maps."""
    bf = ml_dtypes.bfloat16
    f32 = np.float32

    hist = np.asarray(inputs["history"], f32)     # [B, 128, 256]
    act = np.asarray(inputs["action"], f32)       # [B, 128, 256]
    seq = np.concatenate([hist[:, :127], act], axis=1)          # [B, 255, 256]
    seq = np.concatenate(
        [seq, np.zeros((B, 1, DIN), f32)], axis=1)              # [B, 256, 256]

    def gate_perm(W):
        # reference gate order i,f,g,o -> f,i,2g,o along the last axis
        i, f, g, o = (W[..., 0:256], W[..., 256:512],
                      W[..., 512:768], W[..., 768:1024])
        return np.concatenate([f, i, 2.0 * g, o], axis=-1)

    Wk = gate_perm(np.asarray(inputs["Wk"], f32))     # [256, 1024]
    Wrk = gate_perm(np.asarray(inputs["Wrk"], f32))
    bl = gate_perm(np.asarray(inputs["bl"], f32))     # [1024]
    wk_p = np.ascontiguousarray(
        Wk.reshape(2, 128, 1024).transpose(1, 0, 2)).astype(bf)   # [128,2,1024]
    wrk_p = np.ascontiguousarray(
        Wrk.reshape(2, 128, 1024).transpose(1, 0, 2)).astype(bf)
    blt = np.ascontiguousarray(bl.reshape(8, 128).T).astype(f32)  # [128,8]
    Wc = np.asarray(inputs["Wc"], f32)            # [768, 256]
    wc_p = np.ascontiguousarray(
        Wc.reshape(6, 128, 256).transpose(1, 0, 2)).astype(bf)    # [128,6,256]
    Wo = np.asarray(inputs["Wo"], f32)            # [256, 1]
    wo_p = np.ascontiguousarray(
        Wo.reshape(2, 128, 1).transpose(1, 0, 2)).astype(bf)      # [128,2,1]

    def bias2(v, chunks):
        return np.ascontiguousarray(np.asarray(v, f32).reshape(chunks, 128).T)

    shared = {
        "wm": np.asarray(inputs["Wm"], f32).astype(bf),
        "wr": np.asarray(inputs["Wr"], f32).astype(bf),
        "wre": np.asarray(inputs["Wre"], f32).astype(bf),
        "wim": np.asarray(inputs["Wim"], f32).astype(bf),
        "wc": wc_p, "wk": wk_p, "wrk": wrk_p, "wo": wo_p,
        "bm2": bias2(inputs["bm"], 2), "br2": bias2(inputs["br"], 2),
        "bre1": bias2(inputs["bre"], 1), "bim1": bias2(inputs["bim"], 1),
        "bc2": bias2(inputs["bc"], 2),
        "bo1": np.asarray(inputs["bo"], f32).reshape(1, 1),
        "blt": blt,
    }

    mot = np.asarray(inputs["motion_state"], f32)
    rob = np.asarray(inputs["robot_state"], f32)
    real = np.concatenate([np.asarray(inputs["osc_state_real"], f32),
                           np.asarray(inputs["osc_real"], f32)], -1)
    imag = np.concatenate([np.asarray(inputs["osc_state_imag"], f32),
                           np.asarray(inputs["osc_imag"], f32)], -1)

    in_maps = []
    for c in range(NC):
        sl = slice(c * BC, (c + 1) * BC)
        # on-chip col = t*64 + b  (plain t-major)
        sc = seq[sl].reshape(BC, TP, 2, 128)           # [b, t, fk, fp]
        sc = np.ascontiguousarray(sc.transpose(2, 3, 1, 0)).astype(bf)
        m = dict(shared)
        m["seq"] = np.ascontiguousarray(sc.reshape(2, 128, TP * BC))
        m["mot"] = np.ascontiguousarray(mot[sl].T).astype(bf)
        m["rob"] = np.ascontiguousarray(rob[sl].T).astype(bf)
        m["re_"] = np.ascontiguousarray(real[sl].T).astype(bf)
        m["im_"] = np.ascontiguousarray(imag[sl].T).astype(bf)
        in_maps.append(m)
    return in_maps


def kernel(**inputs):
    global _NC_CACHE
    use_bias = bool(np.any(np.asarray(inputs["bl"])))
    if _NC_CACHE is None or _NC_CACHE[1] != use_bias:
        _NC_CACHE = (build_nc(use_bias), use_bias)
    in_maps = _prep_inputs(inputs)
    res = run_bass_kernel_spmd(_NC_CACHE[0], in_maps, core_ids=list(range(NC)))
    out = np.concatenate(
        [np.asarray(res.results[c]["y"], np.float32).T for c in range(NC)], axis=0)
    return out  # [512, 1] float32


# revision 18
# speedup vs baseline: 1.0150x; 1.0150x over previous
"""Trainium2 Bass kernel for nn_Critic (branch MLPs -> 255-step LSTM -> head).

Strategy (hardcoded, 8 cores, data-parallel over batch B=512 -> 64/core):
  - Feature-major on chip: vectors are [feature_chunk(128), batch(64)].
  - bf16 matmul inputs, fp32 PSUM/gates/cell state.
  - PSUM z_all[p, gate, slot, mmcol]: gate-major (weights permuted to
    f,i,g,o order; g rows pre-scaled by 2 so tanh(zg) = 2*sigmoid(2zg)-1).
    Each gate owns 2 banks (8 slots of 128 cols); slot = t mod 8. sigmoid(f)
    for step t can run while the PE still writes i/g/o of the same step
    (different banks), shortening the serial gate chain.
  - Gate chain per step: sig_f -> sig_ig -> sig_o on ACT; on DVE
    tm2 = sf*c, tm1 = (sg-0.5)*si (scalar_tensor_tensor), c' = 2*tm1+tm2
    (scalar_tensor_tensor), then tanh(c') on ACT, h = so*tc on DVE.
  - zx (Wk^T x_t) for step t+4 is emitted right after step t's recurrent
    matmuls: always lands in the opposite bank-half from the sigmoids of
    steps t..t+3, so no PSUM bank serialization, and the PE stays warm
    (no HAM re-throttle). start=True (whole-bank clear) only on slot 0/4
    of each bank, whose other slots hold only dead data at that point.
"""

import os
os.environ.setdefault("TILE_EXHAUSTIVE_MEMORY_SHARE_CHECK", "1")

import numpy as np
import ml_dtypes

import concourse.bass as bass
import concourse.mybir as mybir
import concourse.tile as tile
from concourse import bacc
from concourse.bass_utils import run_bass_kernel_spmd

BF16 = mybir.dt.bfloat16
F32 = mybir.dt.float32
AF = mybir.ActivationFunctionType
ALU = mybir.AluOpType

NC = 8          # cores
B = 512
BC = B // NC    # 64 batch per core
T = 255         # real steps
TP = 256        # padded steps
U = 256
DIN = 256
ND = 5          # PE-warming dummy matmuls (N=512) per step


def build_nc(use_bias=False):
    nc = bacc.Bacc(None, target_bir_lowering=False)

    d_seq = nc.dram_tensor("seq", [2, 128, TP * BC], BF16, kind="ExternalInput")
    d_fe = nc.dram_tensor("fe", [128, 2562], BF16, kind="ExternalInput")
    d_fb = nc.dram_tensor("fb", [128, 17], F32, kind="ExternalInput")
    d_wk = nc.dram_tensor("wk", [128, 2, 1024], BF16, kind="ExternalInput")
    d_wrk = nc.dram_tensor("wrk", [128, 2, 1024], BF16, kind="ExternalInput")
    d_y = nc.dram_tensor("y", [1, BC], F32, kind="ExternalOutput")

    with tile.TileContext(nc) as tc:
        with (
            tc.tile_pool(name="sb", bufs=1) as sb,
            tc.tile_pool(name="rot", bufs=3) as rot,
        ):
            t_wk = sb.tile([128, 2, 1024], BF16, tag="wk")
            t_wrk = sb.tile([128, 2, 1024], BF16, tag="wrk")
            t_fe = sb.tile([128, 2562], BF16, tag="fe")
            t_fb = sb.tile([128, 17], F32, tag="fb")
            t_seq0 = sb.tile([128, TP * BC], BF16, tag="seq0")
            t_seq1 = sb.tile([128, TP * BC], BF16, tag="seq1")
            # slice views into the packed front-end tile
            t_wm = t_fe[0:64, 0:256]
            t_wr = t_fe[:, 256:512]
            t_wre = t_fe[:, 512:640]
            t_wim = t_fe[:, 640:768]
            t_mot = t_fe[0:64, 768:832]
            t_rob = t_fe[:, 832:896]
            t_re = t_fe[:, 896:960]
            t_im = t_fe[:, 960:1024]
            t_h = sb.tile([128, 2 * BC], BF16, tag="h")   # h^T (chunk k at cols k*64)
            t_c = sb.tile([128, 2 * BC], F32, tag="c")    # c^T
            t_z512 = sb.tile([128, 512], BF16, tag="z512")  # zero rhs for PE-warming
            t_cat = sb.tile([128, 6, BC], BF16, tag="cat")
            t_y = sb.tile([1, BC], F32, tag="y")

            # Input DMAs spread across engine queues so the ~0.7us/descriptor
            # issue cost doesn't serialize on one engine. The first seq chunk
            # (steps 0..63) goes first -- it gates zx(0..3).
            CH = 64 * BC
            nc.scalar.dma_start(t_fe[:], d_fe[:])
            nc.scalar.dma_start(t_fb[:], d_fb[:])
            nc.sync.dma_start(t_seq0[:, 0:CH], d_seq[0, :, 0:CH])
            nc.gpsimd.dma_start(t_seq1[:, 0:CH], d_seq[1, :, 0:CH])
            nc.sync.dma_start(t_wrk[:], d_wrk[:])
            nc.gpsimd.dma_start(t_wk[:], d_wk[:])
            for ch in range(1, TP * BC // CH):
                nc.sync.dma_start(
                    t_seq0[:, ch * CH:(ch + 1) * CH], d_seq[0, :, ch * CH:(ch + 1) * CH])
                nc.gpsimd.dma_start(
                    t_seq1[:, ch * CH:(ch + 1) * CH], d_seq[1, :, ch * CH:(ch + 1) * CH])
            t_seq = [t_seq0, t_seq1]
            nc.vector.memset(t_z512[:], 0.0)

            # ---- front-end branch MLPs -> state -> h0, c0 ----
            with tc.tile_pool(name="fp", bufs=1, space="PSUM") as fp:
                p6 = fp.tile([128, 6, BC], F32, tag="p6")
                for m in range(2):
                    nc.tensor.matmul(p6[:, m, :], t_fe[0:64, m * 128:(m + 1) * 128],
                                     t_mot, start=True, stop=True)
                for m in range(2):
                    nc.tensor.matmul(p6[:, 2 + m, :], t_wr[:, m * 128:(m + 1) * 128],
                                     t_rob, start=True, stop=True)
                nc.tensor.matmul(p6[:, 4, :], t_wre, t_re, start=True, stop=True)
                nc.tensor.matmul(p6[:, 5, :], t_wim, t_im, start=True, stop=True)
                for m in range(2):
                    nc.scalar.activation(t_cat[:, m, :], p6[:, m, :], AF.Relu,
                                         bias=t_fb[:, m:m + 1])
                for m in range(2):
                    nc.scalar.activation(t_cat[:, 2 + m, :], p6[:, 2 + m, :], AF.Relu,
                                         bias=t_fb[:, 2 + m:3 + m])
                nc.scalar.activation(t_cat[:, 4, :], p6[:, 4, :], AF.Relu,
                                     bias=t_fb[:, 4:5])
                nc.scalar.activation(t_cat[:, 5, :], p6[:, 5, :], AF.Relu,
                                     bias=t_fb[:, 5:6])
                pst = fp.tile([128, 2, BC], F32, tag="pst")
                for mo in range(2):
                    for kc in range(6):
                        nc.tensor.matmul(
                            pst[:, mo, :],
                            t_fe[:, 1026 + kc * 256 + mo * 128:1026 + kc * 256 + (mo + 1) * 128],
                            t_cat[:, kc, :],
                            start=(kc == 0), stop=(kc == 5))
                for mo in range(2):
                    nc.scalar.activation(t_h[:, mo * BC:(mo + 1) * BC], pst[:, mo, :],
                                         AF.Relu, bias=t_fb[:, 6 + mo:7 + mo])
                    nc.scalar.activation(t_c[:, mo * BC:(mo + 1) * BC], pst[:, mo, :],
                                         AF.Relu, bias=t_fb[:, 6 + mo:7 + mo])

            # ---- LSTM recurrence ----
            with tc.tile_pool(name="zp", bufs=1, space="PSUM") as zp:
                # z_all[p, gate, slot, mm*64+b]: gate order f,i,g,o (weights
                # permuted; g pre-scaled x2). Each gate = 2 banks; slot = t%8.
                z_all = zp.tile([128, 4, 8, 128], F32, tag="zall")

                def emit_zx(tp):
                    # zx for step tp: 16 MMs; start=True (whole-bank clear)
                    # on the first MM into EACH gate's bank at the quad
                    # boundary (tp%4==0) -- that bank's other slots hold only
                    # already-consumed steps then.
                    for g in range(4):
                        for mm in range(2):
                            for k in range(2):
                                st = (tp % 4 == 0 and mm == 0 and k == 0)
                                nc.tensor.matmul(
                                    z_all[:, g, tp % 8, mm * BC:(mm + 1) * BC],
                                    t_wk[:, k, (g * 2 + mm) * 128:(g * 2 + mm + 1) * 128],
                                    t_seq[k][:, tp * BC:(tp + 1) * BC],
                                    start=st, stop=False,
                                    skip_group_check=True)

                def emit_step(t):
                    s = t % 8
                    # recurrent matmuls, gate-major so sigmoid(f) can start
                    # after the first 4 MMs (f banks are done being written)
                    for g in range(4):
                        for mm in range(2):
                            for k in range(2):
                                nc.tensor.matmul(
                                    z_all[:, g, s, mm * BC:(mm + 1) * BC],
                                    t_wrk[:, k, (g * 2 + mm) * 128:(g * 2 + mm + 1) * 128],
                                    t_h[:, k * BC:(k + 1) * BC],
                                    start=False,
                                    stop=(mm == 1 and k == 1),
                                    skip_group_check=True)
                    gbf = rot.tile([128, 128], F32, tag="gbf")    # sigma_f
                    gbig = rot.tile([128, 2, 128], BF16, tag="gbig")  # si, sg
                    gbo = rot.tile([128, 128], BF16, tag="gbo")   # sigma_o
                    tm1 = rot.tile([128, 128], BF16, tag="tm1")
                    tm2 = rot.tile([128, 128], F32, tag="tm2")
                    tmc = rot.tile([128, 128], BF16, tag="tmc")
                    if not use_bias:
                        nc.scalar.activation(gbf[:], z_all[:, 0, s, :],
                                             AF.Sigmoid)
                        nc.scalar.activation(gbig[:], z_all[:, 1:3, s, :],
                                             AF.Sigmoid)
                        nc.scalar.activation(gbo[:], z_all[:, 3, s, :],
                                             AF.Sigmoid)
                    else:
                        # general-bias fallback: per-chunk sigmoids with the
                        # per-partition bias column (g chunks carry 2*bl).
                        for mm in range(2):
                            nc.scalar.activation(
                                gbf[:, mm * BC:(mm + 1) * BC],
                                z_all[:, 0, s, mm * BC:(mm + 1) * BC],
                                AF.Sigmoid, bias=t_fb[:, 8 + mm:9 + mm])
                        for gi in range(2):
                            for mm in range(2):
                                nc.scalar.activation(
                                    gbig[:, gi, mm * BC:(mm + 1) * BC],
                                    z_all[:, 1 + gi, s, mm * BC:(mm + 1) * BC],
                                    AF.Sigmoid,
                                    bias=t_fb[:, 10 + gi * 2 + mm:11 + gi * 2 + mm])
                        for mm in range(2):
                            nc.scalar.activation(
                                gbo[:, mm * BC:(mm + 1) * BC],
                                z_all[:, 3, s, mm * BC:(mm + 1) * BC],
                                AF.Sigmoid, bias=t_fb[:, 14 + mm:15 + mm])
                    # tm2 = sf*c ; tm1 = (sg-0.5)*si ; c' = 2*tm1 + tm2
                    nc.vector.tensor_mul(tm2[:], gbf[:], t_c[:])
                    nc.vector.scalar_tensor_tensor(
                        tm1[:], gbig[:, 1, :], -0.5, gbig[:, 0, :],
                        op0=ALU.add, op1=ALU.mult)
                    nc.vector.scalar_tensor_tensor(
                        t_c[:], tm1[:], 2.0, tm2[:],
                        op0=ALU.mult, op1=ALU.add)
                    nc.scalar.activation(tmc[:], t_c[:], AF.Tanh)
                    nc.vector.tensor_mul(t_h[:], gbo[:], tmc[:])

                    # zx for step t+4 + PE-warming matmuls, emitted AFTER the
                    # gate ops so the scheduler keeps the recurrent MMs and
                    # gate chain tight; these fill the PE during the next
                    # step's gate chain. zx banks are always in the opposite
                    # slot-half from the sigmoid reads of steps t..t+3.
                    # Warming MMs stream a zero rhs with start=False -- adds 0
                    # everywhere (or writes 0 that real zx then accumulates
                    # onto), keeping PE duty high so HAM stays at 8/8.
                    if t + 4 < T:
                        emit_zx(t + 4)
                    H = 4 * (((t + 4) % 8) // 4)
                    for dj in range(ND):
                        nc.tensor.matmul(
                            z_all[:, dj % 4, H:H + 4, :],
                            t_wk[:, 0, (dj % 8) * 128:(dj % 8 + 1) * 128],
                            t_z512[:],
                            start=False, stop=False, skip_group_check=True)

                for tp in range(4):
                    emit_zx(tp)
                for t in range(T):
                    emit_step(t)

            # ---- output head ----
            with tc.tile_pool(name="hp", bufs=1, space="PSUM") as hp:
                py = hp.tile([1, BC], F32, tag="py")
                for k in range(2):
                    nc.tensor.matmul(py[:], t_fe[:, 1024 + k:1025 + k],
                                     t_h[:, k * BC:(k + 1) * BC],
                                     start=(k == 0), stop=(k == 1))
                nc.scalar.activation(t_y[:], py[:], AF.Relu, bias=t_fb[0:1, 16:17])
            nc.sync.dma_start(d_y[:], t_y[:])

    nc.compile()
    return nc


_NC_CACHE = None


def _prep_inputs(inputs):
    """Shard + lay out the full-problem inputs into 8 per-core in_maps."""
    bf = ml_dtypes.bfloat16
    f32 = np.float32

    hist = np.asarray(inputs["history"], f32)     # [B, 128, 256]
    act = np.asarray(inputs["action"], f32)       # [B, 128, 256]
    seq = np.concatenate([hist[:, :127], act], axis=1)          # [B, 255, 256]
    seq = np.concatenate(
        [seq, np.zeros((B, 1, DIN), f32)], axis=1)              # [B, 256, 256]

    def gate_perm(W):
        # reference gate order i,f,g,o -> f,i,2g,o along the last axis
        i, f, g, o = (W[..., 0:256], W[..., 256:512],
                      W[..., 512:768], W[..., 768:1024])
        return np.concatenate([f, i, 2.0 * g, o], axis=-1)

    Wk = gate_perm(np.asarray(inputs["Wk"], f32))     # [256, 1024]
    Wrk = gate_perm(np.asarray(inputs["Wrk"], f32))
    bl = gate_perm(np.asarray(inputs["bl"], f32))     # [1024]
    wk_p = np.ascontiguousarray(
        Wk.reshape(2, 128, 1024).transpose(1, 0, 2)).astype(bf)   # [128,2,1024]
    wrk_p = np.ascontiguousarray(
        Wrk.reshape(2, 128, 1024).transpose(1, 0, 2)).astype(bf)
    blt = np.ascontiguousarray(bl.reshape(8, 128).T).astype(f32)  # [128,8]
    Wc = np.asarray(inputs["Wc"], f32)            # [768, 256]
    wc_p = np.ascontiguousarray(
        Wc.reshape(6, 128, 256).transpose(1, 0, 2)).astype(bf)    # [128,6,256]
    Wo = np.asarray(inputs["Wo"], f32)            # [256, 1]
    wo_p = np.ascontiguousarray(
        Wo.reshape(2, 128, 1).transpose(1, 0, 2)).astype(bf)      # [128,2,1]

    def bias2(v, chunks):
        return np.ascontiguousarray(np.asarray(v, f32).reshape(chunks, 128).T)

    # packed front-end tile [128, 2562] bf16 (see build_nc for the col map)
    fe = np.zeros((128, 2562), f32)
    fe[0:64, 0:256] = np.asarray(inputs["Wm"], f32)
    fe[:, 256:512] = np.asarray(inputs["Wr"], f32)
    fe[:, 512:640] = np.asarray(inputs["Wre"], f32)
    fe[:, 640:768] = np.asarray(inputs["Wim"], f32)
    fe[:, 1024:1026] = wo_p.reshape(128, 2)
    fe[:, 1026:2562] = wc_p.reshape(128, 1536)
    # packed biases [128, 17] f32
    fb = np.zeros((128, 17), f32)
    fb[:, 0:2] = bias2(inputs["bm"], 2)
    fb[:, 2:4] = bias2(inputs["br"], 2)
    fb[:, 4:5] = bias2(inputs["bre"], 1)
    fb[:, 5:6] = bias2(inputs["bim"], 1)
    fb[:, 6:8] = bias2(inputs["bc"], 2)
    fb[:, 8:16] = blt
    fb[0, 16] = np.asarray(inputs["bo"], f32).reshape(())

    shared = {"wk": wk_p, "wrk": wrk_p, "fb": fb}

    mot = np.asarray(inputs["motion_state"], f32)
    rob = np.asarray(inputs["robot_state"], f32)
    real = np.concatenate([np.asarray(inputs["osc_state_real"], f32),
                           np.asarray(inputs["osc_real"], f32)], -1)
    imag = np.concatenate([np.asarray(inputs["osc_state_imag"], f32),
                           np.asarray(inputs["osc_imag"], f32)], -1)

    in_maps = []
    for c in range(NC):
        sl = slice(c * BC, (c + 1) * BC)
        # on-chip col = t*64 + b  (plain t-major)
        sc = seq[sl].reshape(BC, TP, 2, 128)           # [b, t, fk, fp]
        sc = np.ascontiguousarray(sc.transpose(2, 3, 1, 0)).astype(bf)
        m = dict(shared)
        m["seq"] = np.ascontiguousarray(sc.reshape(2, 128, TP * BC))
        fec = fe.copy()
        fec[0:64, 768:832] = mot[sl].T
        fec[:, 832:896] = rob[sl].T
        fec[:, 896:960] = real[sl].T
        fec[:, 960:1024] = imag[sl].T
        m["fe"] = fec.astype(bf)
        in_maps.append(m)
    return in_maps


def kernel(**inputs):
    global _NC_CACHE
    use_bias = bool(np.any(np.asarray(inputs["bl"])))
    if _NC_CACHE is None or _NC_CACHE[1] != use_bias:
        _NC_CACHE = (build_nc(use_bias), use_bias)
    in_maps = _prep_inputs(inputs)
    res = run_bass_kernel_spmd(_NC_CACHE[0], in_maps, core_ids=list(range(NC)))
    out = np.concatenate(
        [np.asarray(res.results[c]["y"], np.float32).T for c in range(NC)], axis=0)
    return out  # [512, 1] float32


# revision 20
# speedup vs baseline: 1.0247x; 1.0096x over previous
"""Trainium2 Bass kernel for nn_Critic (branch MLPs -> 255-step LSTM -> head).

Strategy (hardcoded, 8 cores, data-parallel over batch B=512 -> 64/core):
  - Feature-major on chip: vectors are [feature_chunk(128), batch(64)].
  - bf16 matmul inputs, fp32 PSUM/gates/cell state.
  - PSUM z_all[p, gate, slot, mmcol]: gate-major (weights permuted to
    f,i,g,o order; g rows pre-scaled by 2 so tanh(zg) = 2*sigmoid(2zg)-1).
    Each gate owns 2 banks (8 slots of 128 cols); slot = t mod 8. sigmoid(f)
    for step t can run while the PE still writes i/g/o of the same step
    (different banks), shortening the serial gate chain.
  - Gate chain per step: sig_f -> sig_ig -> sig_o on ACT; on DVE
    tm2 = sf*c, tm1 = (sg-0.5)*si (scalar_tensor_tensor), c' = 2*tm1+tm2
    (scalar_tensor_tensor), then tanh(c') on ACT, h = so*tc on DVE.
  - zx (Wk^T x_t) for step t+4 is emitted right after step t's recurrent
    matmuls: always lands in the opposite bank-half from the sigmoids of
    steps t..t+3, so no PSUM bank serialization, and the PE stays warm
    (no HAM re-throttle). start=True (whole-bank clear) only on slot 0/4
    of each bank, whose other slots hold only dead data at that point.
"""

import os
os.environ.setdefault("TILE_EXHAUSTIVE_MEMORY_SHARE_CHECK", "1")

import numpy as np
import ml_dtypes

import concourse.bass as bass
import concourse.mybir as mybir
import concourse.tile as tile
from concourse import bacc
from concourse.bass_utils import run_bass_kernel_spmd

BF16 = mybir.dt.bfloat16
F32 = mybir.dt.float32
AF = mybir.ActivationFunctionType
ALU = mybir.AluOpType

NC = 8          # cores
B = 512
BC = B // NC    # 64 batch per core
T = 255         # real steps
TP = 256        # padded steps
U = 256
DIN = 256
ND = 5          # PE-warming dummy matmuls (N=512) per step


def build_nc(use_bias=False):
    nc = bacc.Bacc(None, target_bir_lowering=False)

    d_seq = nc.dram_tensor("seq", [2, 128, TP * BC], BF16, kind="ExternalInput")
    d_fe = nc.dram_tensor("fe", [128, 2596], BF16, kind="ExternalInput")
    d_wk = nc.dram_tensor("wk", [128, 2, 1024], BF16, kind="ExternalInput")
    d_wrk = nc.dram_tensor("wrk", [128, 2, 1024], BF16, kind="ExternalInput")
    d_y = nc.dram_tensor("y", [1, BC], F32, kind="ExternalOutput")

    with tile.TileContext(nc) as tc:
        with (
            tc.tile_pool(name="sb", bufs=1) as sb,
            tc.tile_pool(name="rot", bufs=3) as rot,
        ):
            t_wk = sb.tile([128, 2, 1024], BF16, tag="wk")
            t_wrk = sb.tile([128, 2, 1024], BF16, tag="wrk")
            t_fe = sb.tile([128, 2596], BF16, tag="fe")
            t_seq0 = sb.tile([128, TP * BC], BF16, tag="seq0")
            t_seq1 = sb.tile([128, TP * BC], BF16, tag="seq1")
            # slice views into the packed front-end tile; biases are f32
            # bit-packed into the first 34 bf16 cols
            t_fb = t_fe[:, 0:34].bitcast(F32)
            t_wm = t_fe[0:64, 34:290]
            t_wr = t_fe[:, 290:546]
            t_wre = t_fe[:, 546:674]
            t_wim = t_fe[:, 674:802]
            t_mot = t_fe[0:64, 802:866]
            t_rob = t_fe[:, 866:930]
            t_re = t_fe[:, 930:994]
            t_im = t_fe[:, 994:1058]
            t_h = sb.tile([128, 2 * BC], BF16, tag="h")   # h^T (chunk k at cols k*64)
            t_c = sb.tile([128, 2 * BC], F32, tag="c")    # c^T
            t_z512 = sb.tile([128, 512], BF16, tag="z512")  # zero rhs for PE-warming
            t_cat = sb.tile([128, 6, BC], BF16, tag="cat")
            t_y = sb.tile([1, BC], F32, tag="y")

            # Input DMAs spread across engine queues so the ~0.7us/descriptor
            # issue cost doesn't serialize on one engine. The first seq chunk
            # (steps 0..63) goes first -- it gates zx(0..3).
            nc.vector.memset(t_z512[:], 0.0)
            nc.scalar.dma_start(t_fe[:, 0:1058], d_fe[:, 0:1058])
            nc.scalar.dma_start(t_fe[:, 1058:2596], d_fe[:, 1058:2596])
            C0 = 16 * BC
            nc.sync.dma_start(t_seq0[:, 0:C0], d_seq[0, :, 0:C0])
            nc.gpsimd.dma_start(t_seq1[:, 0:C0], d_seq[1, :, 0:C0])
            nc.sync.dma_start(t_wrk[:], d_wrk[:])
            nc.gpsimd.dma_start(t_wk[:], d_wk[:])
            bounds = [16 * BC, 64 * BC, 128 * BC, 192 * BC, 256 * BC]
            for a, b in zip(bounds[:-1], bounds[1:]):
                nc.sync.dma_start(t_seq0[:, a:b], d_seq[0, :, a:b])
                nc.gpsimd.dma_start(t_seq1[:, a:b], d_seq[1, :, a:b])
            t_seq = [t_seq0, t_seq1]
            # dummy sigmoid: force the sigmoid table set (which also contains
            # Relu) to load once, up front, off the critical path
            t_warm = sb.tile([1, 1], F32, tag="warm")
            nc.scalar.activation(t_warm[:], t_z512[0:1, 0:1], AF.Sigmoid)

            # ---- front-end branch MLPs -> state -> h0, c0 ----
            with tc.tile_pool(name="fp", bufs=1, space="PSUM") as fp:
                p6 = fp.tile([128, 6, BC], F32, tag="p6")
                for m in range(2):
                    nc.tensor.matmul(p6[:, m, :], t_fe[0:64, 34 + m * 128:34 + (m + 1) * 128],
                                     t_mot, start=True, stop=True)
                for m in range(2):
                    nc.tensor.matmul(p6[:, 2 + m, :], t_wr[:, m * 128:(m + 1) * 128],
                                     t_rob, start=True, stop=True)
                nc.tensor.matmul(p6[:, 4, :], t_wre, t_re, start=True, stop=True)
                nc.tensor.matmul(p6[:, 5, :], t_wim, t_im, start=True, stop=True)
                for m in range(2):
                    nc.scalar.activation(t_cat[:, m, :], p6[:, m, :], AF.Relu,
                                         bias=t_fb[:, m:m + 1])
                for m in range(2):
                    nc.scalar.activation(t_cat[:, 2 + m, :], p6[:, 2 + m, :], AF.Relu,
                                         bias=t_fb[:, 2 + m:3 + m])
                nc.scalar.activation(t_cat[:, 4, :], p6[:, 4, :], AF.Relu,
                                     bias=t_fb[:, 4:5])
                nc.scalar.activation(t_cat[:, 5, :], p6[:, 5, :], AF.Relu,
                                     bias=t_fb[:, 5:6])
                pst = fp.tile([128, 2, BC], F32, tag="pst")
                for mo in range(2):
                    for kc in range(6):
                        nc.tensor.matmul(
                            pst[:, mo, :],
                            t_fe[:, 1060 + kc * 256 + mo * 128:1060 + kc * 256 + (mo + 1) * 128],
                            t_cat[:, kc, :],
                            start=(kc == 0), stop=(kc == 5))
                for mo in range(2):
                    nc.scalar.activation(t_h[:, mo * BC:(mo + 1) * BC], pst[:, mo, :],
                                         AF.Relu, bias=t_fb[:, 6 + mo:7 + mo])
                    nc.scalar.activation(t_c[:, mo * BC:(mo + 1) * BC], pst[:, mo, :],
                                         AF.Relu, bias=t_fb[:, 6 + mo:7 + mo])

            # ---- LSTM recurrence ----
            with tc.tile_pool(name="zp", bufs=1, space="PSUM") as zp:
                # z_all[p, gate, slot, mm*64+b]: gate order f,i,g,o (weights
                # permuted; g pre-scaled x2). Each gate = 2 banks; slot = t%8.
                z_all = zp.tile([128, 4, 8, 128], F32, tag="zall")

                def emit_zx(tp):
                    # zx for step tp: 16 MMs; start=True (whole-bank clear)
                    # on the first MM into EACH gate's bank at the quad
                    # boundary (tp%4==0) -- that bank's other slots hold only
                    # already-consumed steps then.
                    for g in range(4):
                        for mm in range(2):
                            for k in range(2):
                                st = (tp % 4 == 0 and mm == 0 and k == 0)
                                nc.tensor.matmul(
                                    z_all[:, g, tp % 8, mm * BC:(mm + 1) * BC],
                                    t_wk[:, k, (g * 2 + mm) * 128:(g * 2 + mm + 1) * 128],
                                    t_seq[k][:, tp * BC:(tp + 1) * BC],
                                    start=st, stop=False,
                                    skip_group_check=True)

                def emit_step(t):
                    s = t % 8
                    # recurrent matmuls, gate-major so sigmoid(f) can start
                    # after the first 4 MMs (f banks are done being written)
                    for g in range(4):
                        for mm in range(2):
                            for k in range(2):
                                nc.tensor.matmul(
                                    z_all[:, g, s, mm * BC:(mm + 1) * BC],
                                    t_wrk[:, k, (g * 2 + mm) * 128:(g * 2 + mm + 1) * 128],
                                    t_h[:, k * BC:(k + 1) * BC],
                                    start=False,
                                    stop=(mm == 1 and k == 1),
                                    skip_group_check=True)
                    gbf = rot.tile([128, 128], F32, tag="gbf")    # sigma_f
                    gbig = rot.tile([128, 2, 128], BF16, tag="gbig")  # si, sg
                    gbo = rot.tile([128, 128], BF16, tag="gbo")   # sigma_o
                    tm1 = rot.tile([128, 128], BF16, tag="tm1")
                    tm2 = rot.tile([128, 128], F32, tag="tm2")
                    tmc = rot.tile([128, 128], BF16, tag="tmc")
                    if not use_bias:
                        nc.scalar.activation(gbf[:], z_all[:, 0, s, :],
                                             AF.Sigmoid)
                        nc.scalar.activation(gbig[:], z_all[:, 1:3, s, :],
                                             AF.Sigmoid)
                        nc.scalar.activation(gbo[:], z_all[:, 3, s, :],
                                             AF.Sigmoid)
                    else:
                        # general-bias fallback: per-chunk sigmoids with the
                        # per-partition bias column (g chunks carry 2*bl).
                        for mm in range(2):
                            nc.scalar.activation(
                                gbf[:, mm * BC:(mm + 1) * BC],
                                z_all[:, 0, s, mm * BC:(mm + 1) * BC],
                                AF.Sigmoid, bias=t_fb[:, 8 + mm:9 + mm])
                        for gi in range(2):
                            for mm in range(2):
                                nc.scalar.activation(
                                    gbig[:, gi, mm * BC:(mm + 1) * BC],
                                    z_all[:, 1 + gi, s, mm * BC:(mm + 1) * BC],
                                    AF.Sigmoid,
                                    bias=t_fb[:, 10 + gi * 2 + mm:11 + gi * 2 + mm])
                        for mm in range(2):
                            nc.scalar.activation(
                                gbo[:, mm * BC:(mm + 1) * BC],
                                z_all[:, 3, s, mm * BC:(mm + 1) * BC],
                                AF.Sigmoid, bias=t_fb[:, 14 + mm:15 + mm])
                    # tm2 = sf*c ; tm1 = (sg-0.5)*si ; c' = 2*tm1 + tm2
                    nc.vector.tensor_mul(tm2[:], gbf[:], t_c[:])
                    nc.vector.scalar_tensor_tensor(
                        tm1[:], gbig[:, 1, :], -0.5, gbig[:, 0, :],
                        op0=ALU.add, op1=ALU.mult)
                    nc.vector.scalar_tensor_tensor(
                        t_c[:], tm1[:], 2.0, tm2[:],
                        op0=ALU.mult, op1=ALU.add)
                    nc.scalar.activation(tmc[:], t_c[:], AF.Tanh)
                    nc.vector.tensor_mul(t_h[:], gbo[:], tmc[:])

                    # zx for step t+4 + PE-warming matmuls, emitted AFTER the
                    # gate ops so the scheduler keeps the recurrent MMs and
                    # gate chain tight; these fill the PE during the next
                    # step's gate chain. zx banks are always in the opposite
                    # slot-half from the sigmoid reads of steps t..t+3.
                    # Warming MMs stream a zero rhs with start=False -- adds 0
                    # everywhere (or writes 0 that real zx then accumulates
                    # onto), keeping PE duty high so HAM stays at 8/8.
                    if t + 4 < T:
                        emit_zx(t + 4)
                    H = 4 * (((t + 4) % 8) // 4)
                    for dj in range(ND):
                        nc.tensor.matmul(
                            z_all[:, dj % 4, H:H + 4, :],
                            t_wk[:, 0, (dj % 8) * 128:(dj % 8 + 1) * 128],
                            t_z512[:],
                            start=False, stop=False, skip_group_check=True)

                for tp in range(4):
                    emit_zx(tp)
                for t in range(T):
                    emit_step(t)

            # ---- output head ----
            with tc.tile_pool(name="hp", bufs=1, space="PSUM") as hp:
                py = hp.tile([1, BC], F32, tag="py")
                for k in range(2):
                    nc.tensor.matmul(py[:], t_fe[:, 1058 + k:1059 + k],
                                     t_h[:, k * BC:(k + 1) * BC],
                                     start=(k == 0), stop=(k == 1))
                nc.scalar.activation(t_y[:], py[:], AF.Relu, bias=t_fb[0:1, 16:17])
            nc.sync.dma_start(d_y[:], t_y[:])

    nc.compile()
    return nc


_NC_CACHE = None


def _prep_inputs(inputs):
    """Shard + lay out the full-problem inputs into 8 per-core in_maps."""
    bf = ml_dtypes.bfloat16
    f32 = np.float32

    hist = np.asarray(inputs["history"], f32)     # [B, 128, 256]
    act = np.asarray(inputs["action"], f32)       # [B, 128, 256]
    seq = np.concatenate([hist[:, :127], act], axis=1)          # [B, 255, 256]
    seq = np.concatenate(
        [seq, np.zeros((B, 1, DIN), f32)], axis=1)              # [B, 256, 256]

    def gate_perm(W):
        # reference gate order i,f,g,o -> f,i,2g,o along the last axis
        i, f, g, o = (W[..., 0:256], W[..., 256:512],
                      W[..., 512:768], W[..., 768:1024])
        return np.concatenate([f, i, 2.0 * g, o], axis=-1)

    Wk = gate_perm(np.asarray(inputs["Wk"], f32))     # [256, 1024]
    Wrk = gate_perm(np.asarray(inputs["Wrk"], f32))
    bl = gate_perm(np.asarray(inputs["bl"], f32))     # [1024]
    wk_p = np.ascontiguousarray(
        Wk.reshape(2, 128, 1024).transpose(1, 0, 2)).astype(bf)   # [128,2,1024]
    wrk_p = np.ascontiguousarray(
        Wrk.reshape(2, 128, 1024).transpose(1, 0, 2)).astype(bf)
    blt = np.ascontiguousarray(bl.reshape(8, 128).T).astype(f32)  # [128,8]
    Wc = np.asarray(inputs["Wc"], f32)            # [768, 256]
    wc_p = np.ascontiguousarray(
        Wc.reshape(6, 128, 256).transpose(1, 0, 2)).astype(bf)    # [128,6,256]
    Wo = np.asarray(inputs["Wo"], f32)            # [256, 1]
    wo_p = np.ascontiguousarray(
        Wo.reshape(2, 128, 1).transpose(1, 0, 2)).astype(bf)      # [128,2,1]

    def bias2(v, chunks):
        return np.ascontiguousarray(np.asarray(v, f32).reshape(chunks, 128).T)

    # packed front-end tile [128, 2596] bf16 (see build_nc for the col map);
    # f32 biases bit-packed into the first 34 bf16 cols
    fe = np.zeros((128, 2596), bf)
    fb = np.zeros((128, 17), f32)
    fb[:, 0:2] = bias2(inputs["bm"], 2)
    fb[:, 2:4] = bias2(inputs["br"], 2)
    fb[:, 4:5] = bias2(inputs["bre"], 1)
    fb[:, 5:6] = bias2(inputs["bim"], 1)
    fb[:, 6:8] = bias2(inputs["bc"], 2)
    fb[:, 8:16] = blt
    fb[0, 16] = np.asarray(inputs["bo"], f32).reshape(())
    fe.view(np.uint16)[:, 0:34] = np.ascontiguousarray(fb).view(np.uint16)
    fe[0:64, 34:290] = np.asarray(inputs["Wm"], f32).astype(bf)
    fe[:, 290:546] = np.asarray(inputs["Wr"], f32).astype(bf)
    fe[:, 546:674] = np.asarray(inputs["Wre"], f32).astype(bf)
    fe[:, 674:802] = np.asarray(inputs["Wim"], f32).astype(bf)
    fe[:, 1058:1060] = wo_p.reshape(128, 2)
    fe[:, 1060:2596] = wc_p.reshape(128, 1536)

    shared = {"wk": wk_p, "wrk": wrk_p}

    mot = np.asarray(inputs["motion_state"], f32)
    rob = np.asarray(inputs["robot_state"], f32)
    real = np.concatenate([np.asarray(inputs["osc_state_real"], f32),
                           np.asarray(inputs["osc_real"], f32)], -1)
    imag = np.concatenate([np.asarray(inputs["osc_state_imag"], f32),
                           np.asarray(inputs["osc_imag"], f32)], -1)

    in_maps = []
    for c in range(NC):
        sl = slice(c * BC, (c + 1) * BC)
        # on-chip col = t*64 + b  (plain t-major)
        sc = seq[sl].reshape(BC, TP, 2, 128)           # [b, t, fk, fp]
        sc = np.ascontiguousarray(sc.transpose(2, 3, 1, 0)).astype(bf)
        m = dict(shared)
        m["seq"] = np.ascontiguousarray(sc.reshape(2, 128, TP * BC))
        fec = fe.copy()
        fec[0:64, 802:866] = mot[sl].T.astype(bf)
        fec[:, 866:930] = rob[sl].T.astype(bf)
        fec[:, 930:994] = real[sl].T.astype(bf)
        fec[:, 994:1058] = imag[sl].T.astype(bf)
        m["fe"] = fec
        in_maps.append(m)
    return in_maps


def kernel(**inputs):
    global _NC_CACHE
    use_bias = bool(np.any(np.asarray(inputs["bl"])))
    if _NC_CACHE is None or _NC_CACHE[1] != use_bias:
        _NC_CACHE = (build_nc(use_bias), use_bias)
    in_maps = _prep_inputs(inputs)
    res = run_bass_kernel_spmd(_NC_CACHE[0], in_maps, core_ids=list(range(NC)))
    out = np.concatenate(
        [np.asarray(res.results[c]["y"], np.float32).T for c in range(NC)], axis=0)
    return out  # [512, 1] float32
